# revision 9
# baseline (speedup 1.0000x reference)
"""CrossAttentionN (nn_CrossAttentionN_446676599074) Bass/Tile kernel for TRN2.

Full-input contract: kernel(**inputs) takes the complete tensors, shards them
across 8 NeuronCores (4-way data-parallel over B x 2-way over the per-joint N
stack), runs one SPMD NEFF, and reassembles the full output.

Shapes: x[32,64,22,512], context[32,128,512], Wq[22,512,512], out[32,64,22,512]
Per core: 8 b's, 11 joints, 704 tokens/b. All matmuls in float32r
(~1.4e-4 rel err, 1 cycle/row on the PE at free-dim>=256).
"""
import numpy as np

import concourse.bacc as bacc
import concourse.tile as tile
from concourse import mybir
from concourse.bass_utils import run_bass_kernel_spmd
from concourse.masks import make_identity

F32 = mybir.dt.float32
F32R = mybir.dt.float32r
AF = mybir.ActivationFunctionType

B, T, N, D, H, C = 32, 64, 22, 512, 8, 128
DH = D // H  # 64
BG, NG = 4, 2          # core grid: 4 b-groups x 2 n-groups
BC, NC_ = B // BG, N // NG   # 8 b's, 11 joints per core
NT = NC_ * T           # 704 tokens per b
KC = D // 128          # 4 contraction chunks
FC = D // 128          # 4 output-feature chunks
# token chunks per b: 5 x 128 + 1 x 64
TOK_CHUNKS = [(i * 128, min(128, NT - i * 128)) for i in range((NT + 127) // 128)]


DEBUG = False


def _build():
    nc = bacc.Bacc(None, target_bir_lowering=False)

    x_d = nc.dram_tensor("x", [BC, T, NC_, D], F32, kind="ExternalInput")
    ctx_d = nc.dram_tensor("context", [BC, C, D], F32, kind="ExternalInput")
    wq_d = nc.dram_tensor("Wq", [NC_, D, D], F32, kind="ExternalInput")
    bq_d = nc.dram_tensor("bq", [NC_, D], F32, kind="ExternalInput")
    wk_d = nc.dram_tensor("Wk", [D, D], F32, kind="ExternalInput")
    bk_d = nc.dram_tensor("bk", [D], F32, kind="ExternalInput")
    wv_d = nc.dram_tensor("Wv", [D, D], F32, kind="ExternalInput")
    bv_d = nc.dram_tensor("bv", [D], F32, kind="ExternalInput")
    wo_d = nc.dram_tensor("Wout", [D, D], F32, kind="ExternalInput")
    bo_d = nc.dram_tensor("bout", [D], F32, kind="ExternalInput")
    out_d = nc.dram_tensor("out", [BC, T, NC_, D], F32, kind="ExternalOutput")
    if DEBUG:
        dbg = {
            "d_ctxT": nc.dram_tensor("d_ctxT", [128, KC, BC, C], F32, kind="ExternalOutput"),
            "d_kT": nc.dram_tensor("d_kT", [128, FC, BC, C], F32, kind="ExternalOutput"),
            "d_v": nc.dram_tensor("d_v", [128, BC, D], F32, kind="ExternalOutput"),
            "d_qT": nc.dram_tensor("d_qT", [128, FC, 4, NT], F32, kind="ExternalOutput"),
            "d_expS": nc.dram_tensor("d_expS", [128, NT], F32, kind="ExternalOutput"),
            "d_den": nc.dram_tensor("d_den", [128, 6, H], F32, kind="ExternalOutput"),
            "d_oT": nc.dram_tensor("d_oT", [128, FC, NT], F32, kind="ExternalOutput"),
            "d_onm": nc.dram_tensor("d_onm", [128, 6, D], F32, kind="ExternalOutput"),
            "d_oTnm": nc.dram_tensor("d_oTnm", [128, FC, NT], F32, kind="ExternalOutput"),
        }

    with tile.TileContext(nc) as tc:
        with (
            tc.tile_pool(name="const", bufs=1) as cpool,
            tc.tile_pool(name="kv", bufs=1) as kvpool,
            tc.tile_pool(name="ps", bufs=2, space="PSUM") as ps,
        ):
            # ---- constants / weights ----
            ident = cpool.tile([128, 128], F32)
            make_identity(nc, ident)
            ones1 = cpool.tile([128, 1], F32)
            nc.gpsimd.memset(ones1, 1.0)

            bq_sb = cpool.tile([128, FC, NC_], F32)
            for o in range(FC):
                nc.sync.dma_start(
                    bq_sb[:, o, :], bq_d[:, o * 128 : (o + 1) * 128].transpose([1, 0])
                )
            bk_sb = cpool.tile([128, FC], F32)
            nc.sync.dma_start(bk_sb[:], bk_d.rearrange("(o p) -> p o", p=128))

            row_bv = cpool.tile([1, D], F32)
            nc.sync.dma_start(row_bv[:], bv_d[:].unsqueeze(0))
            bv_bc = cpool.tile([128, D], F32)
            nc.gpsimd.partition_broadcast(bv_bc[:], row_bv[:])
            row_bo = cpool.tile([1, D], F32)
            nc.sync.dma_start(row_bo[:], bo_d[:].unsqueeze(0))
            bo_bc = cpool.tile([128, D], F32)
            nc.gpsimd.partition_broadcast(bo_bc[:], row_bo[:])

            wk_sb = cpool.tile([128, KC, D], F32R)
            nc.gpsimd.dma_start(wk_sb[:], wk_d.rearrange("(kc p) f -> p kc f", p=128))
            wv_sb = cpool.tile([128, KC, D], F32R)
            nc.gpsimd.dma_start(wv_sb[:], wv_d.rearrange("(kc p) f -> p kc f", p=128))
            wo_sb = cpool.tile([128, KC, D], F32R)
            nc.gpsimd.dma_start(wo_sb[:], wo_d.rearrange("(kc p) f -> p kc f", p=128))

            # ---- stage 1: context transpose, K^T, V for all 8 b's ----
            kT = kvpool.tile([128, FC, BC, C], F32R)     # [f_part, fc, b, c]
            v_sb = kvpool.tile([128, BC, D], F32R)       # [c_part, b, f]

            with tc.tile_pool(name="st1", bufs=2) as s1pool:
                ctxT = s1pool.tile([128, KC, BC, C], F32R, bufs=1)  # [d_part, kc, b, c]
                for b in range(BC):
                    ctx_t = s1pool.tile([128, D], F32, tag="ctx")
                    nc.sync.dma_start(ctx_t[:], ctx_d[b])
                    pt = ps.tile([128, 512], F32, tag="t")
                    for kc in range(KC):
                        nc.tensor.transpose(
                            pt[:, kc * 128 : (kc + 1) * 128],
                            ctx_t[:, kc * 128 : (kc + 1) * 128],
                            ident[:],
                        )
                    nc.vector.tensor_copy(
                        ctxT[:, :, b, :],
                        pt.rearrange("p (kc c) -> p kc c", kc=KC),
                    )
                for fc in range(FC):
                    for bh2 in range(2):
                        pk = ps.tile([128, 768], F32, tag="s")
                        for kc in range(KC):
                            nc.tensor.matmul(
                                pk[:, 0:512],
                                wk_sb[:, kc, fc * 128 : (fc + 1) * 128],
                                ctxT[:, kc, bh2 * 4 : bh2 * 4 + 4, :],
                                start=(kc == 0),
                                stop=(kc == KC - 1),
                            )
                        nc.scalar.activation(
                            kT[:, fc, bh2 * 4 : bh2 * 4 + 4, :],
                            pk[:, 0:512].rearrange("p (b c) -> p b c", b=4),
                            AF.Identity,
                            bias=bk_sb[:, fc : fc + 1],
                        )
                if DEBUG:
                    nc.sync.dma_start(dbg["d_ctxT"][:], ctxT[:].bitcast(F32))
                    nc.sync.dma_start(dbg["d_kT"][:], kT[:].bitcast(F32))
                for b in range(BC):
                    pv = ps.tile([128, 768], F32, tag="s")
                    for kc in range(KC):
                        nc.tensor.matmul(
                            pv[:, 0:512],
                            ctxT[:, kc, b, :],
                            wv_sb[:, kc, :],
                            start=(kc == 0),
                            stop=(kc == KC - 1),
                        )
                    nc.vector.tensor_add(v_sb[:, b, :], pv[:, 0:512], bv_bc[:])

            # ---- stages 2+3 per b-half: Q projection then attention ----
            with (
                tc.tile_pool(name="qproj", bufs=1) as qpool,
                tc.tile_pool(name="wqx", bufs=2) as wqpool,
                tc.tile_pool(name="attn", bufs=1) as apool,
                tc.tile_pool(name="eden", bufs=3) as epool,
                tc.tile_pool(name="outp", bufs=3) as opool,
            ):
                for bhalf in range(2):
                    # qT: [f_part, fc, b_local, 704 tokens] tokens contiguous per b
                    qT = qpool.tile([128, FC, 4, NT], F32R, tag="qT")
                    for n in range(NC_):
                        wq_t = wqpool.tile([128, KC, D], F32R, tag="wq")
                        nc.gpsimd.dma_start(
                            wq_t[:], wq_d[n].rearrange("(kc p) f -> p kc f", p=128)
                        )
                        xT = wqpool.tile([128, KC, 256], F32R, tag="xT")
                        for bp in range(2):
                            b0 = bhalf * 4 + bp * 2
                            x_t = wqpool.tile([128, D], F32, tag="x")
                            nc.sync.dma_start(
                                x_t[:],
                                x_d[b0 : b0 + 2, :, n, :].rearrange(
                                    "b t d -> (b t) d"
                                ),
                            )
                            pxt = ps.tile([128, 512], F32, tag="t")
                            for kc in range(KC):
                                nc.tensor.transpose(
                                    pxt[:, kc * 128 : (kc + 1) * 128],
                                    x_t[:, kc * 128 : (kc + 1) * 128],
                                    ident[:],
                                )
                            nc.scalar.copy(
                                xT[:, :, bp * 128 : (bp + 1) * 128],
                                pxt.rearrange("p (kc t) -> p kc t", kc=KC),
                            )
                        for fc in range(FC):
                            pq = ps.tile([128, 768], F32, tag="s")
                            for kc in range(KC):
                                nc.tensor.matmul(
                                    pq[:, 0:256],
                                    wq_t[:, kc, fc * 128 : (fc + 1) * 128],
                                    xT[:, kc, :],
                                    start=(kc == 0),
                                    stop=(kc == KC - 1),
                                )
                            nc.vector.tensor_scalar_add(
                                qT[:, fc, :, n * 64 : (n + 1) * 64],
                                pq[:, 0:256].rearrange("p (b t) -> p b t", b=4),
                                bq_sb[:, fc, n : n + 1],
                            )

                    if DEBUG and bhalf == 0:
                        nc.sync.dma_start(dbg["d_v"][:], v_sb[:].bitcast(F32))
                        nc.sync.dma_start(dbg["d_qT"][:], qT[:].bitcast(F32))
                    # ---- attention for the 4 b's of this half ----
                    for bi in range(4):
                        b = bhalf * 4 + bi
                        oT_un = apool.tile([128, FC, NT], F32, tag="oT_un")
                        den_b = apool.tile([128, len(TOK_CHUNKS), H], F32, tag="den")
                        for h in range(H):
                            hp = (h % 2) * 64
                            fcq = h // 2
                            ps_s = ps.tile([128, 768], F32, tag="s")
                            for c0, cn in [(0, 512), (512, 192)]:
                                nc.tensor.matmul(
                                    ps_s[:, c0 : c0 + cn],
                                    kT[hp : hp + 64, fcq, b, :],
                                    qT[hp : hp + 64, fcq, bi, c0 : c0 + cn],
                                )
                            expS = epool.tile([128, NT], F32R, tag="expS")
                            nc.scalar.activation(
                                expS[:], ps_s[:, 0:NT], AF.Exp, scale=1.0 / 8.0
                            )
                            # denominators, token-major: [cw, 1] per chunk
                            for ti, (t0, cw) in enumerate(TOK_CHUNKS):
                                nc.tensor.matmul(
                                    ps_s[0:cw, NT + ti : NT + ti + 1],
                                    expS[:, t0 : t0 + cw].bitcast(F32),
                                    ones1[:],
                                )
                            po = ps.tile([64, NT], F32, tag="t")
                            for c0, cn in [(0, 512), (512, 192)]:
                                nc.tensor.matmul(
                                    po[:, c0 : c0 + cn],
                                    v_sb[:, b, h * 64 : (h + 1) * 64],
                                    expS[:, c0 : c0 + cn],
                                )
                            if h % 2:
                                nc.vector.tensor_copy(oT_un[hp : hp + 64, fcq, :], po[:])
                            else:
                                nc.scalar.copy(oT_un[hp : hp + 64, fcq, :], po[:])
                            if DEBUG and b == 0 and h == 0:
                                nc.sync.dma_start(dbg["d_expS"][:], expS[:].bitcast(F32))
                            nc.vector.tensor_copy(
                                den_b[:, :, h : h + 1],
                                ps_s[:, NT : NT + len(TOK_CHUNKS)].unsqueeze(2),
                            )
                        if DEBUG and b == 0:
                            nc.sync.dma_start(dbg["d_den"][:], den_b[:])
                            nc.sync.dma_start(dbg["d_oT"][:], oT_un[:])
                        inv_b = apool.tile([128, len(TOK_CHUNKS), H], F32, tag="inv")
                        nc.vector.reciprocal(inv_b[:, 0:5, :], den_b[:, 0:5, :])
                        nc.vector.reciprocal(inv_b[0:64, 5, :], den_b[0:64, 5, :])

                        # transpose -> normalize (token-major) -> transpose back
                        o_nm = apool.tile([128, len(TOK_CHUNKS), D], F32, tag="o_nm")
                        for ti, (t0, cw) in enumerate(TOK_CHUNKS):
                            pt1 = ps.tile([128, 512], F32, tag="t")
                            for fc in range(FC):
                                nc.tensor.transpose(
                                    pt1[0:cw, fc * 128 : (fc + 1) * 128],
                                    oT_un[:, fc, t0 : t0 + cw],
                                    ident[:],
                                )
                            nc.vector.tensor_tensor(
                                o_nm[0:cw, ti].rearrange("p (g f) -> p g f", g=H),
                                pt1[0:cw].rearrange("p (g f) -> p g f", g=H),
                                inv_b[0:cw, ti].unsqueeze(2).to_broadcast([cw, H, DH]),
                                mybir.AluOpType.mult,
                            )
                        if DEBUG and b == 0:
                            nc.sync.dma_start(dbg["d_onm"][:], o_nm[:])
                        oT_nm = apool.tile([128, FC, NT], F32R, tag="oT_nm")
                        for fc in range(FC):
                            pt2 = ps.tile([128, NT], F32, tag="t")
                            for ti, (t0, cw) in enumerate(TOK_CHUNKS):
                                nc.tensor.transpose(
                                    pt2[:, t0 : t0 + cw],
                                    o_nm[0:cw, ti, fc * 128 : (fc + 1) * 128],
                                    ident[0:cw, 0:cw],
                                )
                            nc.scalar.copy(oT_nm[:, fc, :], pt2[:, 0:NT])

                        if DEBUG and b == 0:
                            nc.sync.dma_start(dbg["d_oTnm"][:], oT_nm[:].bitcast(F32))
                        # output projection + bias, then store
                        for ti, (t0, cw) in enumerate(TOK_CHUNKS):
                            po2 = ps.tile([128, 512], F32, tag="t")
                            for fc in range(FC):
                                nc.tensor.matmul(
                                    po2[0:cw, :],
                                    oT_nm[:, fc, t0 : t0 + cw],
                                    wo_sb[:, fc, :],
                                    start=(fc == 0),
                                    stop=(fc == FC - 1),
                                )
                            out_sb = opool.tile([128, D], F32, tag="out")
                            nc.vector.tensor_add(
                                out_sb[0:cw, :], po2[0:cw, :], bo_bc[0:cw, :]
                            )
                            nw = cw // T
                            for k in range(nw):
                                nc.sync.dma_start(
                                    out_d[b, :, 2 * ti + k, :],
                                    out_sb[k * 64 : (k + 1) * 64, :],
                                )

    nc.finalize()
    return nc


_NC_CACHE = None
TRACE = False
TRACE_DIR = None
LAST_EXEC_NS = None


def _get_nc():
    global _NC_CACHE
    if _NC_CACHE is None:
        _NC_CACHE = _build()
    return _NC_CACHE


def make_in_maps(inputs):
    x = np.ascontiguousarray(np.asarray(inputs["x"], dtype=np.float32))
    context = np.ascontiguousarray(np.asarray(inputs["context"], dtype=np.float32))
    Wq = np.ascontiguousarray(np.asarray(inputs["Wq"], dtype=np.float32))
    bq = np.ascontiguousarray(np.asarray(inputs["bq"], dtype=np.float32))
    full = {
        "Wk": np.ascontiguousarray(np.asarray(inputs["Wk"], dtype=np.float32)),
        "bk": np.ascontiguousarray(np.asarray(inputs["bk"], dtype=np.float32)),
        "Wv": np.ascontiguousarray(np.asarray(inputs["Wv"], dtype=np.float32)),
        "bv": np.ascontiguousarray(np.asarray(inputs["bv"], dtype=np.float32)),
        "Wout": np.ascontiguousarray(np.asarray(inputs["Wout"], dtype=np.float32)),
        "bout": np.ascontiguousarray(np.asarray(inputs["bout"], dtype=np.float32)),
    }

    in_maps = []
    for core in range(8):
        bg, ng = core // NG, core % NG
        bs, ns = slice(bg * BC, (bg + 1) * BC), slice(ng * NC_, (ng + 1) * NC_)
        m = {
            "x": np.ascontiguousarray(x[bs, :, ns, :]),
            "context": np.ascontiguousarray(context[bs]),
            "Wq": np.ascontiguousarray(Wq[ns]),
            "bq": np.ascontiguousarray(bq[ns]),
        }
        m.update(full)
        in_maps.append(m)
    return in_maps


def kernel(**inputs) -> np.ndarray:
    in_maps = make_in_maps(inputs)
    nc = _get_nc()
    kwargs = {}
    if TRACE:
        kwargs = dict(trace=True, trace_cores=[0], tmpdir=TRACE_DIR)
    res = run_bass_kernel_spmd(nc, in_maps, core_ids=list(range(8)), **kwargs)
    global LAST_EXEC_NS
    LAST_EXEC_NS = res.exec_time_ns

    out = np.empty((B, T, N, D), dtype=np.float32)
    for core in range(8):
        bg, ng = core // NG, core % NG
        out[bg * BC : (bg + 1) * BC, :, ng * NC_ : (ng + 1) * NC_, :] = res.results[
            core
        ]["out"]
    return out


if __name__ == "__main__":
    rng = np.random.default_rng(0)
    s = 0.02
    ins = {
        "x": rng.standard_normal((B, T, N, D), dtype=np.float32),
        "context": rng.standard_normal((B, C, D), dtype=np.float32),
        "Wq": rng.standard_normal((N, D, D), dtype=np.float32) * s,
        "bq": rng.standard_normal((N, D), dtype=np.float32) * s,
        "Wk": rng.standard_normal((D, D), dtype=np.float32) * s,
        "bk": rng.standard_normal((D,), dtype=np.float32) * s,
        "Wv": rng.standard_normal((D, D), dtype=np.float32) * s,
        "bv": rng.standard_normal((D,), dtype=np.float32) * s,
        "Wout": rng.standard_normal((D, D), dtype=np.float32) * s,
        "bout": rng.standard_normal((D,), dtype=np.float32) * s,
    }
    out = kernel(**ins)
    print("kernel out", out.shape, out.dtype, float(np.abs(out).mean()))
